# revision 1
# baseline (speedup 1.0000x reference)
"""Trainium2 Bass kernel v2: multi-head attention (B=2, T=2048, C=2048, H=16, D=128).

Sharding: tensor-parallel over heads. 8 cores x 2 heads each.
  - W_qkv columns sliced per head-pair, W_proj rows sliced per head-pair.
  - Each core computes a partial output [B*T, C]; host sums the 8 partials.

v2 changes vs baseline (465us):
  - Per-batch activation buffers + interleaved emission so the Tile
    scheduler overlaps attn(b0) with proj(b1) and attn(b1) with
    outproj(b0): PE never waits on the exp pipeline.
  - Score blocks paired [128, 2, 512] (2 PSUM banks) -> one exp
    ACTIVATE per 1024 columns (halves ACT instruction overhead).
  - Softmax reciprocal via ScalarE ln -> exp(scale=-1) (shares the
    natural_log_exp table set with the scores exp) instead of the
    3.3us partition-serial DVE reciprocal.
  - dacc accumulated on 1024-wide bf16 tiles; denominator contraction
    via two ones-matmuls straight off dacc (no fold).
  - h-batched rope (doubled cos/sin tables) to cut DVE op count.
  - copies pinned: v-copies + half of out-copies on ScalarE, rest DVE.
"""

import math

import numpy as np

N_CORES = 8
B, T, C = 2, 2048, 2048
N_HEAD, D = 16, 128
HPC = N_HEAD // N_CORES          # heads per core
JC = HPC * D                     # per-core slice width of qkv/proj dims

RP = 256                         # proj token tile (moving free dim)
RT = 512                         # attention query tile
KB = 128                         # key block (contraction tile)

# rope table dtype: "bf16" saves SBUF + DMA, needs mixed-dtype TT support
TABLE_DT = "f32"


def _build(Bp, Tp, Cp, hpc, d):
    import concourse.bacc as bacc
    import concourse.tile as tile
    from concourse import mybir

    f32 = mybir.dt.float32
    bf16 = mybir.dt.bfloat16
    Exp = mybir.ActivationFunctionType.Exp
    Ln = mybir.ActivationFunctionType.Ln
    Copy = mybir.ActivationFunctionType.Copy

    jc = hpc * d
    BT = Bp * Tp
    n_ck = Cp // 128             # contraction chunks for proj
    n_rt = Tp // RP              # proj token tiles per batch
    n_sub = RP // 128            # v sub-blocks per proj tile
    n_kb = Tp // KB              # key blocks per batch
    n_kbp = n_kb // 2            # key-block pairs
    n_qt = Tp // RT              # query tiles per batch
    n_rb = Tp // 128             # row blocks for out proj
    n_ot = Cp // RT              # output column tiles
    scale = 1.0 / math.sqrt(d)
    hd = d // 2
    tdt = bf16 if TABLE_DT == "bf16" else f32

    nc = bacc.Bacc("TRN2", target_bir_lowering=False, debug=False)

    xT = nc.declare_dram_parameter("xT", [Cp, BT], bf16, isOutput=False)
    wqkv = nc.declare_dram_parameter("wqkv", [Cp, 3 * jc], bf16,
                                     isOutput=False)
    wp = nc.declare_dram_parameter("wp", [jc, Cp], bf16, isOutput=False)
    ones_d = nc.declare_dram_parameter("ones", [128, 1], bf16, isOutput=False)
    cosT = nc.declare_dram_parameter("cosT", [d, Tp], tdt, isOutput=False)
    sinT = nc.declare_dram_parameter("sinT", [d, Tp], tdt, isOutput=False)
    out = nc.declare_dram_parameter("out", [BT, Cp], bf16, isOutput=True)

    with tile.TileContext(nc) as tc:
        with (
            nc.allow_low_precision(reason="bf16 staging, f32 PSUM accum"),
            tc.tile_pool(name="wpool", bufs=1) as wpool,
            tc.tile_pool(name="acts", bufs=1) as acts,
            tc.tile_pool(name="xpool", bufs=17) as xpool,
            tc.tile_pool(name="rope", bufs=4) as ropep,
            tc.tile_pool(name="epool", bufs=4) as epool,
            tc.tile_pool(name="dpool", bufs=3) as dpool,
            tc.tile_pool(name="small", bufs=2) as small,
            tc.tile_pool(name="opool", bufs=6) as opool,
            tc.tile_pool(name="pss", bufs=2, space="PSUM") as pss,
            tc.tile_pool(name="psy", bufs=1, space="PSUM") as psy,
        ):
            psp_cm = tc.tile_pool(name="psp", bufs=1, space="PSUM")
            psp = psp_cm.__enter__()
            psoA_cm = tc.tile_pool(name="psoA", bufs=1, space="PSUM")
            pso = psoA_cm.__enter__()
            # ---- resident weights / tables ----
            # inputs (w, x) stream on the sync/SP queue; tables + wp + all
            # output traffic go on the scalar queue so they never starve
            # the x-tile stream.
            wq_sb, wk_sb, wv_sb = [], [], []
            xpair = {}
            for ck in range(n_ck):
                t = wpool.tile([128, 3 * jc], bf16, tag=f"w{ck}",
                               name=f"w{ck}")
                nc.sync.dma_start(t, wqkv[ck * 128:(ck + 1) * 128, :])
                wq_sb.append(t[:, 0:jc])
                wk_sb.append(t[:, jc:2 * jc])
                wv_sb.append(t[:, 2 * jc:3 * jc])
                # interleave the (b0, rt0+rt1) x prefetch 1:1 with the w
                # chunks so proj(rt0) is fed chunk-by-chunk from t=0
                xp = xpool.tile([128, 2 * RP], bf16, tag="xt",
                                name=f"xtpre{ck}")
                nc.sync.dma_start(xp, xT[ck * 128:(ck + 1) * 128, 0:2 * RP])
                xpair[(0, 0, ck)] = xp
            ones_sb = wpool.tile([128, 1], bf16, tag="ones")
            nc.sync.dma_start(ones_sb, ones_d[:])
            # doubled rope tables for h-batched rope: [d, hpc, Tp]
            cos2 = wpool.tile([d, hpc, Tp], tdt, tag="cos2")
            sin2 = wpool.tile([d, hpc, Tp], tdt, tag="sin2")
            for h in range(hpc):
                nc.scalar.dma_start(cos2[:, h, :], cosT[:])
                nc.scalar.dma_start(sin2[:, h, :], sinT[:])
            wp_sb = wpool.tile([128, hpc, Cp], bf16, tag="wp")
            nc.scalar.dma_start(wp_sb, wp.rearrange("(h p) o -> p h o", p=128))

            # per-batch activation tensors (both batches resident -> the
            # scheduler can overlap attn(b) with proj(b+1))
            qT_sb = [acts.tile([128, hpc, Tp], bf16, tag=f"qT{b}",
                               name=f"qT{b}") for b in range(Bp)]
            kT_sb = [acts.tile([128, hpc, Tp], bf16, tag=f"kT{b}",
                               name=f"kT{b}") for b in range(Bp)]
            v_sb = [acts.tile([128, n_kb, jc], bf16, tag=f"v{b}",
                              name=f"v{b}") for b in range(Bp)]
            yT_sb = [acts.tile([128, hpc, Tp], bf16, tag=f"yT{b}",
                               name=f"yT{b}") for b in range(Bp)]

            def proj_tile_part(b, rt, part, state, n_parts=4):
                """qkv projection for one RP-wide token tile + rope,
                emitted in ck-range parts so attention work can weave
                between them at fine grain."""
                tsl = slice(rt * RP, (rt + 1) * RP)
                if part == 0:
                    state["q"] = psp.tile([128, hpc, RP], f32, tag="qps",
                                          name=f"qps{b}_{rt}")
                    state["k"] = psp.tile([128, hpc, RP], f32, tag="kps",
                                          name=f"kps{b}_{rt}")
                    state["v"] = psp.tile([128, n_sub, jc], f32, tag="vps",
                                          name=f"vps{b}_{rt}")
                q_ps, k_ps, v_ps = state["q"], state["k"], state["v"]
                cpp = n_ck // n_parts
                for ck in range(part * cpp, (part + 1) * cpp):
                    # x streamed in [128, 2*RP] pair-slabs (1KB descriptors)
                    pk = (b, rt // 2, ck)
                    if pk not in xpair:
                        xp = xpool.tile([128, 2 * RP], bf16, tag="xt")
                        p0 = b * Tp + (rt // 2) * 2 * RP
                        nc.sync.dma_start(
                            xp, xT[ck * 128:(ck + 1) * 128, p0:p0 + 2 * RP])
                        xpair[pk] = xp
                    xt = xpair[pk][:, (rt % 2) * RP:(rt % 2) * RP + RP]
                    first = ck == 0
                    last = ck == n_ck - 1
                    for h in range(hpc):
                        nc.tensor.matmul(
                            q_ps[:, h, :],
                            wq_sb[ck][:, h * d:(h + 1) * d],
                            xt, start=(first and h == 0),
                            stop=(last and h == hpc - 1),
                            skip_group_check=True)
                        nc.tensor.matmul(
                            k_ps[:, h, :],
                            wk_sb[ck][:, h * d:(h + 1) * d],
                            xt, start=(first and h == 0),
                            stop=(last and h == hpc - 1),
                            skip_group_check=True)
                    for s in range(n_sub):
                        nc.tensor.matmul(
                            v_ps[:, s, :],
                            xt[:, s * 128:(s + 1) * 128],
                            wv_sb[ck], start=(first and s == 0),
                            stop=(last and s == n_sub - 1),
                            skip_group_check=True)
                if part != n_parts - 1:
                    return
                # h-batched rope epilogue on [128, hpc*RP]:
                #   dst = psum*cos2 + swap(psum)*sin2_signed
                for ps, dst in ((q_ps, qT_sb[b]), (k_ps, kT_sb[b])):
                    c2 = cos2[:, :, tsl]
                    s2 = sin2[:, :, tsl]
                    t1 = ropep.tile([d, hpc, RP], f32, tag="t1")
                    nc.vector.tensor_mul(t1, ps, c2)
                    t2 = ropep.tile([d, hpc, RP], f32, tag="t2")
                    nc.vector.tensor_mul(t2[0:hd], ps[hd:d], s2[0:hd])
                    nc.vector.tensor_mul(t2[hd:d], ps[0:hd], s2[hd:d])
                    nc.vector.tensor_add(dst[:, :, tsl], t1, t2)
                # v copy (both sub-blocks in one ACT instruction)
                nc.scalar.activation(
                    v_sb[b][:, rt * n_sub:(rt + 1) * n_sub, :], v_ps, Copy)

            def proj_tile(b, rt):
                st = {}
                for part in range(4):
                    proj_tile_part(b, rt, part, st)

            def attn_unit(b, qt, kb, h, y_ps, daccs, equads, pend):
                """one (query-tile, key-block, head) attention step.

                s tiles are single-bank [128, RT], double-buffered -> the
                next MM1 overlaps the current exp.  e tiles are quads
                [128, 4, RT] shared by 4 consecutive key blocks so dacc
                accumulates 2048 elems per DVE op.
                """
                qsl = slice(qt * RT, (qt + 1) * RT)
                s_ps = pss.tile([128, RT], f32, tag="s")
                nc.tensor.matmul(
                    s_ps,
                    kT_sb[b][:, h, kb * KB:(kb + 1) * KB],
                    qT_sb[b][:, h, qsl],
                    start=True, stop=True, skip_group_check=True)
                if kb % 4 == 0:
                    equads[h] = epool.tile([128, 4, RT], bf16, tag="e",
                                           name=f"e{b}_{qt}_{h}_{kb}")
                eq = equads[h]
                nc.scalar.activation(eq[:, kb % 4, :], s_ps, Exp,
                                     scale=scale)
                if kb % 4 == 3:
                    qd = kb // 4
                    if qd == 0:
                        nc.vector.tensor_copy(out=daccs[h], in_=eq)
                    else:
                        nc.vector.tensor_add(daccs[h], daccs[h], eq)
                pend[h].append((kb, eq))

            def mm2_drain(b, h, y_ps, pend, keep):
                while len(pend[h]) > keep:
                    kb, eq = pend[h].pop(0)
                    nc.tensor.matmul(
                        y_ps[h],
                        v_sb[b][:, kb, h * d:(h + 1) * d],
                        eq[:, kb % 4, :],
                        start=(kb == 0), stop=(kb == n_kb - 1),
                        skip_group_check=True)

            def finalize(b, qt, h, y_ps, daccs):
                qsl = slice(qt * RT, (qt + 1) * RT)
                dsum = pso.tile([1, RT], f32, tag="fin", bufs=1,
                                name=f"dsum{b}_{qt}_{h}")
                for i in range(4):
                    nc.tensor.matmul(dsum, ones_sb, daccs[h][:, i, :],
                                     start=(i == 0), stop=(i == 3),
                                     skip_group_check=True)
                # reciprocal: spread the RT denominators across 16
                # partitions (DMA reshape) so the DVE iterative divide is
                # 32 elems/lane instead of RT on one lane; no ACT tables.
                ds_sb = small.tile([1, RT], f32, tag="ds")
                if b == 0:
                    nc.scalar.activation(ds_sb, dsum, Copy)
                else:
                    nc.vector.tensor_copy(out=ds_sb, in_=dsum)
                d16 = small.tile([16, RT // 16], f32, tag="d16")
                nc.scalar.dma_start(d16, ds_sb)
                r16 = small.tile([16, RT // 16], f32, tag="r16")
                nc.vector.reciprocal(r16, d16)
                rec = small.tile([1, RT], f32, tag="rec")
                nc.scalar.dma_start(rec, r16)
                bc = small.tile([128, RT], f32, tag="bc")
                nc.gpsimd.partition_broadcast(out_ap=bc, in_ap=rec)
                nc.vector.tensor_mul(yT_sb[b][:, h, qsl], y_ps[h], bc)

            def attn_qt(b, qt, interleave=None):
                """all attention work for one query tile; interleave is a
                list of thunks emitted between key-block pairs."""
                y_ps = [psy.tile([d, RT], f32, tag=f"y{h}",
                                 name=f"y{b}_{qt}_{h}")
                        for h in range(hpc)]
                daccs = [dpool.tile([128, 4, RT], bf16, tag="dacc",
                                    name=f"dacc{b}_{qt}_{h}")
                         for h in range(hpc)]
                equads = [None] * hpc
                pend = [[] for _ in range(hpc)]
                il = list(interleave or [])
                # spread the filler thunks across the kb loop
                points = {}
                for i, th in enumerate(il):
                    points.setdefault(min(n_kb - 1, i * n_kb // len(il)),
                                      []).append(th)
                for kb in range(n_kb):
                    for h in range(hpc):
                        attn_unit(b, qt, kb, h, y_ps, daccs, equads, pend)
                        # lag the MM2s two key-blocks behind their exp so
                        # the PE never head-of-line blocks on ScalarE
                        mm2_drain(b, h, y_ps, pend, 2)
                    for th in points.get(kb, []):
                        th()
                for h in range(hpc):
                    mm2_drain(b, h, y_ps, pend, 0)
                for h in range(hpc):
                    finalize(b, qt, h, y_ps, daccs)

            def outproj_unit(b, rb, ot, eng):
                o_ps = pso.tile([128, RT], f32, tag="o")
                for h in range(hpc):
                    nc.tensor.matmul(
                        o_ps,
                        yT_sb[b][:, h, rb * 128:(rb + 1) * 128],
                        wp_sb[:, h, ot * RT:(ot + 1) * RT],
                        start=(h == 0), stop=(h == hpc - 1),
                        skip_group_check=True)
                o_sb = opool.tile([128, RT], bf16, tag="o")
                if eng == 0:
                    nc.vector.tensor_copy(out=o_sb, in_=o_ps)
                else:
                    nc.scalar.activation(o_sb, o_ps, Copy)
                nc.scalar.dma_start(
                    out[b * Tp + rb * 128:b * Tp + (rb + 1) * 128,
                        ot * RT:(ot + 1) * RT],
                    o_sb)

            # ================= emission schedule =================
            ppb = n_rt // n_qt          # proj tiles per query tile
            opb = n_rb // n_qt          # row blocks per query tile
            cnt = [0]
            # P1: proj b0 with attn(b0, qt0) streaming kb-wise behind
            # the tiles that produce its k/v blocks (fills the otherwise
            # idle ScalarE and covers DMA stalls with PE work)
            yq0 = [psy.tile([d, RT], f32, tag=f"y{h}", name=f"yq0_{h}")
                   for h in range(hpc)]
            dq0 = [dpool.tile([128, 4, RT], bf16, tag="dacc",
                              name=f"daccq0_{h}") for h in range(hpc)]
            eq0 = [None] * hpc
            pq0 = [[] for _ in range(hpc)]
            for rt in range(n_rt):
                proj_tile(0, rt)
                if rt >= 2:
                    for kb in (2 * (rt - 2), 2 * (rt - 2) + 1):
                        for h in range(hpc):
                            attn_unit(0, 0, kb, h, yq0, dq0, eq0, pq0)
                            mm2_drain(0, h, yq0, pq0, 2)
            # finish qt0's remaining key blocks, then finalize it
            for kb in range(2 * (n_rt - 2), n_kb):
                for h in range(hpc):
                    attn_unit(0, 0, kb, h, yq0, dq0, eq0, pq0)
                    mm2_drain(0, h, yq0, pq0, 2)
            for h in range(hpc):
                mm2_drain(0, h, yq0, pq0, 0)
            for h in range(hpc):
                finalize(0, 0, h, yq0, dq0)
            # P2: attn b0 qt1-3 interleaved with ALL 8 proj-b1 tiles
            # (quarter-tile weave, distributed across the 3 query tiles)
            for qt in range(1, n_qt):
                j = qt - 1
                thunks = []
                for rt in range(j * n_rt // 3, (j + 1) * n_rt // 3):
                    st = {}
                    for part in range(4):
                        thunks.append(
                            lambda rt=rt, part=part, st=st:
                                proj_tile_part(1, rt, part, st))
                attn_qt(0, qt, interleave=thunks)
            # proj + b0 dsums done -> release 4 banks for the P3 pool
            psoA_cm.__exit__(None, None, None)
            psp_cm.__exit__(None, None, None)
            psoB_cm = tc.tile_pool(name="psoB", bufs=3, space="PSUM")
            pso = psoB_cm.__enter__()
            # P3: attn b1; outproj b0 and b1 woven in 2-unit thunks so
            # ScalarE never starves between query tiles
            def op_thunks(units):
                ths = []
                for i in range(0, len(units), 2):
                    chunk = units[i:i + 2]
                    def th(chunk=chunk):
                        for b_, rb, ot in chunk:
                            cnt[0] += 1
                            outproj_unit(b_, rb, ot, 0)
                    ths.append(th)
                return ths
            for qt in range(n_qt):
                units = [(0, rb, ot)
                         for rb in range(qt * opb, (qt + 1) * opb)
                         for ot in range(n_ot)]
                if qt > 0:
                    units += [(1, rb, ot)
                              for rb in range((qt - 1) * opb, qt * opb)
                              for ot in range(n_ot)]
                attn_qt(1, qt, interleave=op_thunks(units))
            # P4: last query tile's outproj b1
            for rb in range((n_qt - 1) * opb, n_qt * opb):
                for ot in range(n_ot):
                    cnt[0] += 1
                    outproj_unit(1, rb, ot, 0)
            psoB_cm.__exit__(None, None, None)

    nc.compile()
    return nc


def _prep_in_maps(x, cos, sin, W_qkv, W_proj, n_cores, hpc, d):
    """Host-side shard prep: pure layout work (transpose / slice / sign fold)."""
    Bp, Tp, Cp = x.shape
    jc = hpc * d
    import ml_dtypes
    tdt = ml_dtypes.bfloat16 if TABLE_DT == "bf16" else np.float32
    xTa = np.ascontiguousarray(x.reshape(Bp * Tp, Cp).T).astype(ml_dtypes.bfloat16)
    cosT = np.ascontiguousarray(cos.T).astype(tdt)
    sinT = np.ascontiguousarray(sin.T).copy()
    sinT[: d // 2] *= -1.0
    sinT = sinT.astype(tdt)
    in_maps = []
    for c in range(n_cores):
        j0, j1 = c * jc, (c + 1) * jc
        in_maps.append({
            "xT": xTa,
            "wqkv": np.ascontiguousarray(np.concatenate(
                [W_qkv[:, j0:j1], W_qkv[:, Cp + j0:Cp + j1],
                 W_qkv[:, 2 * Cp + j0:2 * Cp + j1]], axis=1,
            )).astype(ml_dtypes.bfloat16),
            "wp": np.ascontiguousarray(W_proj[j0:j1, :]).astype(ml_dtypes.bfloat16),
            "ones": np.ones((128, 1), dtype=ml_dtypes.bfloat16),
            "cosT": cosT,
            "sinT": sinT,
        })
    return in_maps


def _install_ntff_hook():
    """Enable NTFF profiling under axon when the boot image lacks the
    antenv.axon_hooks shim. Harmless if anything is missing."""
    import sys
    import types
    try:
        from antenv.axon_hooks import get_axon_ntff_profile_hook
        if get_axon_ntff_profile_hook() is not None:
            return
    except ImportError:
        pass
    try:
        sys.path.insert(0, "/root/.axon_site")
        from trn_agent_boot.trn_boot import _ntff_profile_via_ctypes

        hook = _ntff_profile_via_ctypes("/opt/axon/libaxon_pjrt.so")
        if hook is None:
            return
        mod = types.ModuleType("antenv.axon_hooks")
        mod.get_axon_ntff_profile_hook = lambda: hook
        mod.set_axon_ntff_profile_hook = lambda h: None
        import antenv
        antenv.axon_hooks = mod
        sys.modules["antenv.axon_hooks"] = mod
    except Exception:
        pass


def _run(x, cos, sin, W_qkv, W_proj, trace=False):
    from concourse.bass_utils import run_bass_kernel_spmd

    if trace:
        _install_ntff_hook()

    x = np.ascontiguousarray(x, dtype=np.float32)
    cos = np.ascontiguousarray(cos, dtype=np.float32)
    sin = np.ascontiguousarray(sin, dtype=np.float32)
    W_qkv = np.ascontiguousarray(W_qkv, dtype=np.float32)
    W_proj = np.ascontiguousarray(W_proj, dtype=np.float32)

    Bp, Tp, Cp = x.shape
    nc = _build(Bp, Tp, Cp, HPC, D)
    in_maps = _prep_in_maps(x, cos, sin, W_qkv, W_proj, N_CORES, HPC, D)
    res = run_bass_kernel_spmd(nc, in_maps, core_ids=list(range(N_CORES)),
                               trace=trace)
    acc = np.zeros((Bp * Tp, Cp), dtype=np.float32)
    for i in range(N_CORES):
        acc += np.asarray(res.results[i]["out"], dtype=np.float32)
    return acc.reshape(Bp, Tp, Cp), res


def kernel(x, cos, sin, W_qkv, W_proj):
    out, _ = _run(x, cos, sin, W_qkv, W_proj, trace=False)
    return out

